# revision 14
# baseline (speedup 1.0000x reference)
"""Trainium2 Bass kernel for nn_NeuralTensorDiagLayer.

Computes out = tanh(concat([e1, e2], -1) @ V + diag + b) where
diag[k] = (sum_b(e1*e2) @ W[k]) / (B*D), broadcast over batch.

Sharding (8 NeuronCores, 2D: 4 batch groups x 2 k_out halves):
  - Core c handles batch rows [1024*(c//2), 1024*(c//2+1)) and k_out
    columns [1024*(c%2), 1024*(c%2+1)).
  - All main-path tensors are cast to bf16 on the host (V from
    uniform(-1,1), x = concat(e1,e2) transposed): rel-err budget is 2e-2
    and bf16 end-to-end measures ~1e-2, while halving HBM traffic and
    keeping the TensorEngine at 1 col/cycle.
  - x^T and V stream into SBUF fully resident via interleaved [128,1024]
    DMAs (2 KiB lines) ordered so contraction tile j (x1_j, x2_j, v_j,
    v_{16+j}) lands early; the main matmul's first PSUM group chases the
    DMA stream and the rest runs from SBUF at full rate.
  - Main matmul: 3 PSUM groups of (3,3,2) k-tiles x 2 batch-half banks.
    Groups 0/1 drain PSUM->stage with DVE/ScalarE copies split per bank;
    group 2 is tanh'ed directly out of PSUM (ScalarE reads PSUM).
  - diag: per-tile fused mul+reduce partials on DVE as x tiles arrive,
    8-core AllReduce of s=[128,16] (each batch row counted twice -> 0.5
    folded into DIAG_SCALE), then a 256-col diag slice as 16 f32r
    matmuls (N=256 -> 1 cycle/row) pinned AFTER main group 1 in the
    TensorE stream (AllReduce is long done by then; pinning avoids the
    baseline's 17us TensorE stall), AllGather over [[0,2,4,6],[1,3,5,7]]
    assembles each k_out half (diag slice index sc = (c%2)*4 + c//2 is
    applied host-side so the device program stays SPMD-identical).
  - tanh+bias on ScalarE with diag as per-partition bias, fp32 out tiles
    DMA'd per k-tile ([k_out, batch] transposed); host reassembles.
"""

import os
import sys

for _p in ("/opt/trn_rl_repo", "/root/.axon_site/_ro/trn_rl_repo"):
    if os.path.isdir(_p) and _p not in sys.path:
        sys.path.append(_p)

import numpy as np

N_CORES = 8
B, D, K_OUT = 4096, 2048, 2048
FEAT = 2 * D
BG, KH = 4, 2                 # batch groups x kout halves
BPC = B // BG                 # 1024 batch rows per core
KHC = K_OUT // KH             # 1024 kout cols per core
KPC = K_OUT // N_CORES        # 256 diag rows per core
FT = FEAT // 128              # 32 feature tiles
DT = D // 128                 # 16 e1-space feature tiles
KTL = KHC // 128              # 8 local kout tiles
KGROUPS = (3, 3, 2)           # kout tile groups (2*g PSUM banks each)
DIAG_SCALE = 0.5 / (B * D)    # 0.5: the 8-core allreduce double-counts rows

_CACHE = {}


def _build_nc():
    import concourse.bacc as bacc
    import concourse.tile as tile
    import concourse.mybir as mybir
    from concourse.tile_rust import add_dep_helper

    dt = mybir.dt
    nc = bacc.Bacc("TRN2", target_bir_lowering=False, debug=False,
                   num_devices=N_CORES)

    xt = nc.dram_tensor("xt", [FEAT, BPC], dt.bfloat16, kind="ExternalInput").ap()
    v = nc.dram_tensor("v", [FEAT, KHC], dt.bfloat16, kind="ExternalInput").ap()
    wt = nc.dram_tensor("wt", [128, D * KPC // 128], dt.bfloat16,
                        kind="ExternalInput").ap()
    bvec = nc.dram_tensor("bvec", [1, KPC], dt.float32, kind="ExternalInput").ap()
    out = nc.dram_tensor("out", [KHC, BPC], dt.bfloat16, kind="ExternalOutput").ap()

    core_ids = list(range(N_CORES))
    ag_groups = [[0, 2, 4, 6], [1, 3, 5, 7]]

    with tile.TileContext(nc) as tc:
        with tc.tile_pool(name="xpool", bufs=1) as xpool, \
             tc.tile_pool(name="vpool", bufs=1) as vpool, \
             tc.tile_pool(name="wpool", bufs=1) as wpool, \
             tc.tile_pool(name="spool", bufs=1) as spool, \
             tc.tile_pool(name="scratch", bufs=2) as scratch, \
             tc.tile_pool(name="stage", bufs=1) as stage_pool, \
             tc.tile_pool(name="opool", bufs=2) as opool, \
             tc.tile_pool(name="psum", bufs=6, space="PSUM") as pp, \
             tc.tile_pool(name="psd", bufs=1, space="PSUM") as ppd, \
             tc.tile_pool(name="dram", bufs=1, space="DRAM") as dram:

            # ---- interleaved resident loads ----
            # All HWDGE DMAs drain ONE FIFO queue in issue order, so issue
            # exactly in the main loop's consumption order: j-step j needs
            # (x tile j, v tile j). Granularity ramps up (singles -> pairs
            # -> quads) so the first matmuls start ~5us earlier while later
            # transfers stay big. The diag path needs e2 tiles (x tiles
            # 16..31) too, but only by ~mid-kernel, which the paired order
            # delivers anyway.
            x_all = xpool.tile([128, FT * BPC], dt.bfloat16)
            v_all = vpool.tile([128, FT * KHC], dt.bfloat16)

            def multi_load(dst_tile, dst_cols, src_t, tile0, n):
                nc.sync.dma_start(
                    dst_tile[:, tile0 * dst_cols:(tile0 + n) * dst_cols]
                    .rearrange("p (j c) -> p j c", j=n),
                    src_t[tile0 * 128:(tile0 + n) * 128, :]
                    .rearrange("(j p) c -> p j c", p=128))

            for t in range(2):                      # singles: j = 0, 1
                multi_load(x_all, BPC, xt, t, 1)
                multi_load(v_all, KHC, v, t, 1)
            for t in range(1, 4):                   # pairs: j = 2..7
                multi_load(x_all, BPC, xt, 2 * t, 2)
                multi_load(v_all, KHC, v, 2 * t, 2)
            for b in range(2, 8):                   # quads: j = 8..31
                multi_load(x_all, BPC, xt, 4 * b, 4)
                multi_load(v_all, KHC, v, 4 * b, 4)
            # diag-path weights, host-prepacked to [128, DT*KPC] so this is
            # one flat 8KB-per-partition transfer (needed only mid-kernel)
            wt_sb = wpool.tile([128, DT * KPC], dt.bfloat16)
            nc.sync.dma_start(wt_sb[:], wt[:])
            b_sb = spool.tile([1, KPC], dt.float32, name="b_sb")
            nc.sync.dma_start(b_sb[:], bvec[:])

            # ---- diag partials as x-tile pairs arrive: mul on DVE, the
            # ---- batch-sum via ScalarE Copy+accum (keeps DVE light) ----
            s_sb = spool.tile([128, DT], dt.float32)
            trash = scratch.tile([128, BPC], dt.bfloat16, name="trash")
            for j in range(DT):
                prod = scratch.tile([128, BPC], dt.bfloat16, tag="prod",
                                    name=f"prod{j}")
                nc.vector.tensor_mul(
                    prod[:],
                    x_all[:, j * BPC:(j + 1) * BPC],
                    x_all[:, (DT + j) * BPC:(DT + j + 1) * BPC])
                nc.scalar.activation(trash[:], prod[:],
                                     mybir.ActivationFunctionType.Copy,
                                     accum_out=s_sb[:, j:j + 1])

            # ---- AllReduce s over all cores (8 KiB) ----
            s_in = dram.tile([128, DT], dt.float32)
            s_out = dram.tile([128, DT], dt.float32, addr_space="Shared")
            nc.sync.dma_start(s_in[:], s_sb[:])
            nc.gpsimd.collective_compute(
                "AllReduce", mybir.AluOpType.add,
                replica_groups=[core_ids],
                ins=[s_in.opt()], outs=[s_out.opt()])
            s_r = spool.tile([128, DT], dt.float32, name="s_r")
            nc.sync.dma_start(s_r[:], s_out[:])

            # ---- main matmul: out^T = V_half^T @ x^T, bf16 on TensorE ----
            n_staged = KGROUPS[0] + KGROUPS[1]
            n_last = KGROUPS[2]
            stage = stage_pool.tile([128, n_staged * BPC], dt.float32,
                                    name="stage")
            diag_cols = spool.tile([128, KTL], dt.float32, name="diag_cols")
            k0 = 0
            for kg, g in enumerate(KGROUPS):
                last = kg == len(KGROUPS) - 1
                pss = [[pp.tile([128, 512], dt.float32, tag="ps",
                                name=f"ps{kg}_{q}_{b2}")
                        for b2 in range(2)] for q in range(g)]
                for j in range(FT):
                    for q in range(g):
                        for b2 in range(2):
                            mm = nc.tensor.matmul(
                                pss[q][b2][:],
                                v_all[:, j * KHC + (k0 + q) * 128:
                                      j * KHC + (k0 + q + 1) * 128],
                                x_all[:, j * BPC + b2 * 512:
                                      j * BPC + (b2 + 1) * 512],
                                start=(j == 0), stop=(j == FT - 1))
                    if last and j == 8:
                        # ---- diag slice [1, KPC] = s @ wt, bf16 matmuls,
                        # pinned deep enough into the TensorE stream that the
                        # ~50us AllReduce is finished by the time the stream
                        # reaches them ----
                        s_bf = spool.tile([128, DT], dt.bfloat16, name="s_bf")
                        nc.vector.tensor_copy(s_bf[:], s_r[:])
                        ps_d = ppd.tile([1, KPC], dt.float32)
                        for jd in range(DT):
                            dmm = nc.tensor.matmul(
                                ps_d[:],
                                s_bf[:, jd:jd + 1],
                                wt_sb[:, jd * KPC:(jd + 1) * KPC],
                                start=(jd == 0), stop=(jd == DT - 1))
                            if jd == 0:
                                add_dep_helper(mm.ins, dmm.ins, sync=False,
                                               reason="diag mms mid last group")
                        diag_sb = spool.tile([1, KPC], dt.float32,
                                             name="diag_sb")
                        nc.vector.tensor_scalar_mul(diag_sb[:], ps_d[:],
                                                    DIAG_SCALE)
                        nc.vector.tensor_add(diag_sb[:], diag_sb[:], b_sb[:])

                        # ---- AllGather diag within kout-half subgroup ----
                        d_in = dram.tile([1, KPC], dt.float32, name="d_in")
                        d_out = dram.tile([KTL, 128], dt.float32, name="d_out")
                        nc.sync.dma_start(d_in[:], diag_sb[:])
                        nc.gpsimd.collective_compute(
                            "AllGather", mybir.AluOpType.bypass,
                            replica_groups=ag_groups,
                            ins=[d_in.opt()], outs=[d_out.opt()])
                        # [128, KTL]: partition p, col k <- half[k*128 + p]
                        nc.sync.dma_start(diag_cols[:],
                                          d_out[:].rearrange("k p -> p k"))

                        # ---- tanh for staged groups (overlaps last group) --
                        for kt in range(n_staged):
                            ot = opool.tile([128, BPC], dt.bfloat16, tag="ot",
                                            name=f"ot{kt}")
                            nc.scalar.activation(
                                ot[:], stage[:, kt * BPC:(kt + 1) * BPC],
                                mybir.ActivationFunctionType.Tanh,
                                bias=diag_cols[:, kt:kt + 1])
                            nc.sync.dma_start(out[kt * 128:(kt + 1) * 128, :],
                                              ot[:])
                if not last:
                    # drain PSUM -> stage, banks split across DVE/ScalarE
                    for q in range(g):
                        kt = k0 + q
                        for b2 in range(2):
                            dst = stage[:, kt * BPC + b2 * 512:
                                        kt * BPC + (b2 + 1) * 512]
                            if b2 == 0:
                                nc.vector.tensor_copy(dst, pss[q][b2][:])
                            else:
                                nc.scalar.activation(
                                    dst, pss[q][b2][:],
                                    mybir.ActivationFunctionType.Copy)
                else:
                    # last group: tanh straight out of PSUM (ScalarE)
                    for q in range(g):
                        kt = k0 + q
                        ot2 = opool.tile([128, BPC], dt.bfloat16, tag="ot",
                                         name=f"ot_last_{q}")
                        for b2 in range(2):
                            nc.scalar.activation(
                                ot2[:, b2 * 512:(b2 + 1) * 512],
                                pss[q][b2][:],
                                mybir.ActivationFunctionType.Tanh,
                                bias=diag_cols[:, kt:kt + 1])
                        nc.sync.dma_start(out[kt * 128:(kt + 1) * 128, :],
                                          ot2[:])
                k0 += g

    nc.compile()
    return nc


def _get_nc():
    if "nc" not in _CACHE:
        _CACHE["nc"] = _build_nc()
    return _CACHE["nc"]


def make_in_maps(e1, e2, W, V, b):
    import ml_dtypes
    bf16 = ml_dtypes.bfloat16

    in_maps = []
    for c in range(N_CORES):
        g, h = c // 2, c % 2
        sc = h * 4 + g            # permuted diag-slice index (see module doc)
        rows = slice(g * BPC, (g + 1) * BPC)
        krows = slice(sc * KPC, (sc + 1) * KPC)
        xt = np.ascontiguousarray(
            np.concatenate([e1[rows], e2[rows]], axis=1).T).astype(bf16)
        in_maps.append({
            "xt": xt,
            "v": np.ascontiguousarray(V[:, h * KHC:(h + 1) * KHC]).astype(bf16),
            "wt": np.ascontiguousarray(
                W[krows].T.reshape(16, 128, 256).transpose(1, 0, 2)
                .reshape(128, 4096)).astype(bf16),
            "bvec": b[krows].reshape(1, KPC),
        })
    return in_maps


def kernel(e1, e2, W, V, b):
    from concourse.bass_utils import run_bass_kernel_spmd

    e1 = np.asarray(e1, dtype=np.float32)
    e2 = np.asarray(e2, dtype=np.float32)
    W = np.asarray(W, dtype=np.float32)
    V = np.asarray(V, dtype=np.float32)
    b = np.asarray(b, dtype=np.float32)

    nc = _get_nc()
    res = run_bass_kernel_spmd(nc, make_in_maps(e1, e2, W, V, b),
                               list(range(N_CORES)))
    out = np.empty((B, K_OUT), dtype=np.float32)
    for c in range(N_CORES):
        g, h = c // 2, c % 2
        out[g * BPC:(g + 1) * BPC, h * KHC:(h + 1) * KHC] = \
            res.results[c]["out"].T.astype(np.float32)
    return out


# revision 15
# speedup vs baseline: 1.0579x; 1.0579x over previous
"""Trainium2 Bass kernel for nn_NeuralTensorDiagLayer.

Computes out = tanh(concat([e1, e2], -1) @ V + diag + b) where
diag[k] = (sum_b(e1*e2) @ W[k]) / (B*D), broadcast over batch.

Sharding (8 NeuronCores, 2D: 4 batch groups x 2 k_out halves):
  - Core c handles batch rows [1024*(c//2), 1024*(c//2+1)) and k_out
    columns [1024*(c%2), 1024*(c%2+1)).
  - All main-path tensors are cast to bf16 on the host (V from
    uniform(-1,1), x = concat(e1,e2) transposed): rel-err budget is 2e-2
    and bf16 end-to-end measures ~1e-2, while halving HBM traffic and
    keeping the TensorEngine at 1 col/cycle.
  - x^T and V stream into SBUF fully resident via interleaved [128,1024]
    DMAs (2 KiB lines) ordered so contraction tile j (x1_j, x2_j, v_j,
    v_{16+j}) lands early; the main matmul's first PSUM group chases the
    DMA stream and the rest runs from SBUF at full rate.
  - Main matmul: 3 PSUM groups of (3,3,2) k-tiles x 2 batch-half banks.
    Groups 0/1 drain PSUM->stage with DVE/ScalarE copies split per bank;
    group 2 is tanh'ed directly out of PSUM (ScalarE reads PSUM).
  - diag: per-tile fused mul+reduce partials on DVE as x tiles arrive,
    8-core AllReduce of s=[128,16] (each batch row counted twice -> 0.5
    folded into DIAG_SCALE), then a 256-col diag slice as 16 f32r
    matmuls (N=256 -> 1 cycle/row) pinned AFTER main group 1 in the
    TensorE stream (AllReduce is long done by then; pinning avoids the
    baseline's 17us TensorE stall), AllGather over [[0,2,4,6],[1,3,5,7]]
    assembles each k_out half (diag slice index sc = (c%2)*4 + c//2 is
    applied host-side so the device program stays SPMD-identical).
  - tanh+bias on ScalarE with diag as per-partition bias, fp32 out tiles
    DMA'd per k-tile ([k_out, batch] transposed); host reassembles.
"""

import os
import sys

for _p in ("/opt/trn_rl_repo", "/root/.axon_site/_ro/trn_rl_repo"):
    if os.path.isdir(_p) and _p not in sys.path:
        sys.path.append(_p)

import numpy as np

N_CORES = 8
B, D, K_OUT = 4096, 2048, 2048
FEAT = 2 * D
BG, KH = 4, 2                 # batch groups x kout halves
BPC = B // BG                 # 1024 batch rows per core
KHC = K_OUT // KH             # 1024 kout cols per core
KPC = K_OUT // N_CORES        # 256 diag rows per core
FT = FEAT // 128              # 32 feature tiles
DT = D // 128                 # 16 e1-space feature tiles
KTL = KHC // 128              # 8 local kout tiles
KGROUPS = (3, 3, 2)           # kout tile groups (2*g PSUM banks each)
DIAG_SCALE = 0.5 / (B * D)    # 0.5: the 8-core allreduce double-counts rows

_CACHE = {}


def _build_nc():
    import concourse.bacc as bacc
    import concourse.tile as tile
    import concourse.mybir as mybir
    from concourse.tile_rust import add_dep_helper

    dt = mybir.dt
    nc = bacc.Bacc("TRN2", target_bir_lowering=False, debug=False,
                   num_devices=N_CORES)

    xt = nc.dram_tensor("xt", [FEAT, BPC], dt.bfloat16, kind="ExternalInput").ap()
    v = nc.dram_tensor("v", [FEAT, KHC], dt.bfloat16, kind="ExternalInput").ap()
    wt = nc.dram_tensor("wt", [128, D * KPC // 128], dt.bfloat16,
                        kind="ExternalInput").ap()
    bvec = nc.dram_tensor("bvec", [1, KPC], dt.float32, kind="ExternalInput").ap()
    out = nc.dram_tensor("out", [KHC, BPC], dt.bfloat16, kind="ExternalOutput").ap()

    core_ids = list(range(N_CORES))
    ag_groups = [[0, 2, 4, 6], [1, 3, 5, 7]]

    with tile.TileContext(nc) as tc:
        with tc.tile_pool(name="xpool", bufs=1) as xpool, \
             tc.tile_pool(name="vpool", bufs=1) as vpool, \
             tc.tile_pool(name="wpool", bufs=1) as wpool, \
             tc.tile_pool(name="spool", bufs=1) as spool, \
             tc.tile_pool(name="scratch", bufs=2) as scratch, \
             tc.tile_pool(name="stage", bufs=1) as stage_pool, \
             tc.tile_pool(name="opool", bufs=2) as opool, \
             tc.tile_pool(name="psum", bufs=6, space="PSUM") as pp, \
             tc.tile_pool(name="psd", bufs=1, space="PSUM") as ppd, \
             tc.tile_pool(name="dram", bufs=1, space="DRAM") as dram:

            # ---- interleaved resident loads ----
            # All HWDGE DMAs drain ONE FIFO queue in issue order, so issue
            # exactly in the main loop's consumption order: j-step j needs
            # (x tile j, v tile j). Granularity ramps up (singles -> pairs
            # -> quads) so the first matmuls start ~5us earlier while later
            # transfers stay big. The diag path needs e2 tiles (x tiles
            # 16..31) too, but only by ~mid-kernel, which the paired order
            # delivers anyway.
            x_all = xpool.tile([128, FT * BPC], dt.bfloat16)
            v_all = vpool.tile([128, FT * KHC], dt.bfloat16)

            def multi_load(dst_tile, dst_cols, src_t, tile0, n):
                nc.sync.dma_start(
                    dst_tile[:, tile0 * dst_cols:(tile0 + n) * dst_cols]
                    .rearrange("p (j c) -> p j c", j=n),
                    src_t[tile0 * 128:(tile0 + n) * 128, :]
                    .rearrange("(j p) c -> p j c", p=128))

            for t in range(2):                      # singles: j = 0, 1
                multi_load(x_all, BPC, xt, t, 1)
                multi_load(v_all, KHC, v, t, 1)
            for t in range(1, 4):                   # pairs: j = 2..7
                multi_load(x_all, BPC, xt, 2 * t, 2)
                multi_load(v_all, KHC, v, 2 * t, 2)
            for b in range(2, 8):                   # quads: j = 8..31
                multi_load(x_all, BPC, xt, 4 * b, 4)
                multi_load(v_all, KHC, v, 4 * b, 4)
            # diag-path weights, host-prepacked to [128, DT*KPC] so this is
            # one flat 8KB-per-partition transfer (needed only mid-kernel)
            wt_sb = wpool.tile([128, DT * KPC], dt.bfloat16)
            nc.sync.dma_start(wt_sb[:], wt[:])
            b_sb = spool.tile([1, KPC], dt.float32, name="b_sb")
            nc.sync.dma_start(b_sb[:], bvec[:])

            # ---- diag partials as x-tile pairs arrive: mul on DVE, the
            # ---- batch-sum via ScalarE Copy+accum (keeps DVE light) ----
            s_sb = spool.tile([128, DT], dt.float32)
            trash = scratch.tile([128, BPC], dt.bfloat16, name="trash")
            for j in range(DT):
                prod = scratch.tile([128, BPC], dt.bfloat16, tag="prod",
                                    name=f"prod{j}")
                nc.vector.tensor_mul(
                    prod[:],
                    x_all[:, j * BPC:(j + 1) * BPC],
                    x_all[:, (DT + j) * BPC:(DT + j + 1) * BPC])
                nc.scalar.activation(trash[:], prod[:],
                                     mybir.ActivationFunctionType.Copy,
                                     accum_out=s_sb[:, j:j + 1])

            # ---- AllReduce s over all cores (8 KiB) ----
            s_in = dram.tile([128, DT], dt.float32)
            s_out = dram.tile([128, DT], dt.float32, addr_space="Shared")
            nc.sync.dma_start(s_in[:], s_sb[:])
            nc.gpsimd.collective_compute(
                "AllReduce", mybir.AluOpType.add,
                replica_groups=[core_ids],
                ins=[s_in.opt()], outs=[s_out.opt()])
            s_r = spool.tile([128, DT], dt.float32, name="s_r")
            nc.sync.dma_start(s_r[:], s_out[:])

            # ---- main matmul: out^T = V_half^T @ x^T, bf16 on TensorE ----
            n_staged = KGROUPS[0] + KGROUPS[1]
            n_last = KGROUPS[2]
            stage = stage_pool.tile([128, n_staged * BPC], dt.float32,
                                    name="stage")
            diag_cols = spool.tile([128, KTL], dt.float32, name="diag_cols")
            k0 = 0
            for kg, g in enumerate(KGROUPS):
                last = kg == len(KGROUPS) - 1
                pss = [[pp.tile([128, 512], dt.float32, tag="ps",
                                name=f"ps{kg}_{q}_{b2}")
                        for b2 in range(2)] for q in range(g)]
                for j in range(FT):
                    for q in range(g):
                        for b2 in range(2):
                            mm = nc.tensor.matmul(
                                pss[q][b2][:],
                                v_all[:, j * KHC + (k0 + q) * 128:
                                      j * KHC + (k0 + q + 1) * 128],
                                x_all[:, j * BPC + b2 * 512:
                                      j * BPC + (b2 + 1) * 512],
                                start=(j == 0), stop=(j == FT - 1))
                    if kg == 1 and j == 24:
                        # ---- diag slice [1, KPC] = s @ wt, bf16 matmuls,
                        # pinned deep enough into the TensorE stream that the
                        # ~50us AllReduce is finished by the time the stream
                        # reaches them, and early enough that the AllGather
                        # completes while the last group still computes ----
                        s_bf = spool.tile([128, DT], dt.bfloat16, name="s_bf")
                        nc.vector.tensor_copy(s_bf[:], s_r[:])
                        ps_d = ppd.tile([1, KPC], dt.float32)
                        for jd in range(DT):
                            dmm = nc.tensor.matmul(
                                ps_d[:],
                                s_bf[:, jd:jd + 1],
                                wt_sb[:, jd * KPC:(jd + 1) * KPC],
                                start=(jd == 0), stop=(jd == DT - 1))
                            if jd == 0:
                                add_dep_helper(mm.ins, dmm.ins, sync=False,
                                               reason="diag mms mid group 1")
                        diag_sb = spool.tile([1, KPC], dt.float32,
                                             name="diag_sb")
                        nc.vector.tensor_scalar_mul(diag_sb[:], ps_d[:],
                                                    DIAG_SCALE)
                        nc.vector.tensor_add(diag_sb[:], diag_sb[:], b_sb[:])

                        # ---- AllGather diag within kout-half subgroup ----
                        d_in = dram.tile([1, KPC], dt.float32, name="d_in")
                        d_out = dram.tile([KTL, 128], dt.float32, name="d_out")
                        nc.sync.dma_start(d_in[:], diag_sb[:])
                        nc.gpsimd.collective_compute(
                            "AllGather", mybir.AluOpType.bypass,
                            replica_groups=ag_groups,
                            ins=[d_in.opt()], outs=[d_out.opt()])
                        # [128, KTL]: partition p, col k <- half[k*128 + p]
                        nc.sync.dma_start(diag_cols[:],
                                          d_out[:].rearrange("k p -> p k"))

                if not last:
                    # drain PSUM -> stage; group 1 goes entirely on DVE so
                    # the ScalarE queue can't block on the diag-gated tanhs
                    for q in range(g):
                        kt = k0 + q
                        for b2 in range(2):
                            dst = stage[:, kt * BPC + b2 * 512:
                                        kt * BPC + (b2 + 1) * 512]
                            if kg == 0 and b2 == 1:
                                nc.scalar.activation(
                                    dst, pss[q][b2][:],
                                    mybir.ActivationFunctionType.Copy)
                            else:
                                nc.vector.tensor_copy(dst, pss[q][b2][:])
                    if kg == 1:
                        # tanh for all staged tiles: runs as soon as the
                        # AllGather lands, overlapping the last group
                        for kt in range(n_staged):
                            ot = opool.tile([128, BPC], dt.bfloat16, tag="ot",
                                            name=f"ot{kt}")
                            nc.scalar.activation(
                                ot[:], stage[:, kt * BPC:(kt + 1) * BPC],
                                mybir.ActivationFunctionType.Tanh,
                                bias=diag_cols[:, kt:kt + 1])
                            nc.sync.dma_start(out[kt * 128:(kt + 1) * 128, :],
                                              ot[:])
                else:
                    # last group: tanh straight out of PSUM (ScalarE)
                    for q in range(g):
                        kt = k0 + q
                        ot2 = opool.tile([128, BPC], dt.bfloat16, tag="ot",
                                         name=f"ot_last_{q}")
                        for b2 in range(2):
                            nc.scalar.activation(
                                ot2[:, b2 * 512:(b2 + 1) * 512],
                                pss[q][b2][:],
                                mybir.ActivationFunctionType.Tanh,
                                bias=diag_cols[:, kt:kt + 1])
                        nc.sync.dma_start(out[kt * 128:(kt + 1) * 128, :],
                                          ot2[:])
                k0 += g

    nc.compile()
    return nc


def _get_nc():
    if "nc" not in _CACHE:
        _CACHE["nc"] = _build_nc()
    return _CACHE["nc"]


def make_in_maps(e1, e2, W, V, b):
    import ml_dtypes
    bf16 = ml_dtypes.bfloat16

    in_maps = []
    for c in range(N_CORES):
        g, h = c // 2, c % 2
        sc = h * 4 + g            # permuted diag-slice index (see module doc)
        rows = slice(g * BPC, (g + 1) * BPC)
        krows = slice(sc * KPC, (sc + 1) * KPC)
        xt = np.ascontiguousarray(
            np.concatenate([e1[rows], e2[rows]], axis=1).T).astype(bf16)
        in_maps.append({
            "xt": xt,
            "v": np.ascontiguousarray(V[:, h * KHC:(h + 1) * KHC]).astype(bf16),
            "wt": np.ascontiguousarray(
                W[krows].T.reshape(16, 128, 256).transpose(1, 0, 2)
                .reshape(128, 4096)).astype(bf16),
            "bvec": b[krows].reshape(1, KPC),
        })
    return in_maps


def kernel(e1, e2, W, V, b):
    from concourse.bass_utils import run_bass_kernel_spmd

    e1 = np.asarray(e1, dtype=np.float32)
    e2 = np.asarray(e2, dtype=np.float32)
    W = np.asarray(W, dtype=np.float32)
    V = np.asarray(V, dtype=np.float32)
    b = np.asarray(b, dtype=np.float32)

    nc = _get_nc()
    res = run_bass_kernel_spmd(nc, make_in_maps(e1, e2, W, V, b),
                               list(range(N_CORES)))
    out = np.empty((B, K_OUT), dtype=np.float32)
    for c in range(N_CORES):
        g, h = c // 2, c % 2
        out[g * BPC:(g + 1) * BPC, h * KHC:(h + 1) * KHC] = \
            res.results[c]["out"].T.astype(np.float32)
    return out
